# revision 9
# baseline (speedup 1.0000x reference)
"""Trainium2 Bass kernel for nn_Attn: batched column-softmax attention energies.

Math (per batch element b):
    E = encoder_outputs[:, b, :]            # [H, T]
    d = decoder_hidden[b]                   # [H]
    s = E^T d                               # [T]  (scores)
    w[h, t] = E[h, t] * s[t]
    sm = softmax over h of w (per column t)
    out[b, h] = sum_t sm[h, t]

Implementation (per core, data parallel over batch: 8 cores x 8 batch):
    - Load E in natural layout [h_part, t_free] (DMA-efficient: t contiguous).
    - d replicated across partitions once per b (gpsimd partition_broadcast).
    - Per t-chunk j: 8 PE transpose-mode matmuls -> PSUM tile Et [t_part, 1024h]
      (fp32, exact).
    - DVE pass 1 (tensor_tensor_reduce): accum_add(Et * d_bcast) = scores s_col
      [128t, 1] directly in per-partition layout.
    - DVE pass 2 (tensor_tensor_reduce): out = -(Et * s_bcast),
      accum_min = -max(s*Et) = mneg.
    - One ACT op: e = Exp(s_col * Et + mneg) read straight from PSUM, written
      as bf16, accum_out z = sum_h e  (z >= 1 by construction -> no div by 0).
    - DVE reciprocal r = 1/z (+ bf16 cast); PE matmul lhsT=r_bf16:
      out[1, h] += sum_t r_t e[t, h] accumulated over t-chunks in PSUM
      (softmax normalization folded into the matmul).
"""

import numpy as np

import concourse.bass as bass
import concourse.mybir as mybir
from concourse.bass_utils import run_bass_kernel_spmd
from concourse.tile import TileContext

H = 1024
B = 64
T = 1024
N_CORES = 8
B_LOC = B // N_CORES  # 8 batch elements per core
NHC = H // 128        # 8 h-chunks
NTC = T // 128        # 8 t-chunks

F32 = mybir.dt.float32
BF16 = mybir.dt.bfloat16


def _split_waits(nc, max_waits=1):
    """Workaround for this container's walrus: control/compute instructions
    accept only one sync-wait command. Hoist extra waits onto single-wait
    Drain carriers inserted just before the instruction (same engine)."""
    n_new = 0
    for f in nc.m.functions:
        for blk in f.blocks:
            new_insts = []
            for inst in blk.instructions:
                si = inst.sync_info
                if si is not None and si.on_wait is not None and len(si.on_wait) > max_waits:
                    waits = list(si.on_wait)
                    while len(waits) > max_waits:
                        w = waits.pop(0)
                        d = mybir.InstDrain(
                            name=f"I-ws-{nc.next_id()}", ins=[], outs=[]
                        )
                        d.engine = inst.engine
                        d.sync_info = mybir.SyncInfo(on_wait=[w], on_update=[])
                        new_insts.append(d)
                        n_new += 1
                    si.on_wait = waits
                new_insts.append(inst)
            blk.instructions = new_insts
    return n_new


def build_program(final_dt=BF16, split_waits=True):
    """Build the single-core Bass/Tile program (same program runs SPMD on 8 cores)."""
    nc = bass.Bass("TRN2", debug=False, num_devices=N_CORES)
    enc_h = nc.dram_tensor("enc", [H, B_LOC, T], F32, kind="ExternalInput")
    dec_h = nc.dram_tensor("dec", [B_LOC, H], F32, kind="ExternalInput")
    ident_h = nc.dram_tensor("ident", [128, 128], F32, kind="ExternalInput")
    out_h = nc.dram_tensor("out", [B_LOC, H], F32, kind="ExternalOutput")

    enc = enc_h.ap()
    dec = dec_h.ap()
    ident = ident_h.ap()
    out = out_h.ap()

    AF = mybir.ActivationFunctionType
    OP = mybir.AluOpType

    with TileContext(nc) as tc:
        with (
            tc.tile_pool(name="const", bufs=1) as constp,
            tc.tile_pool(name="natp", bufs=4) as natp,
            tc.tile_pool(name="junkp", bufs=2) as junkp,
            tc.tile_pool(name="ep", bufs=3) as ep,
            tc.tile_pool(name="dbp", bufs=2) as dbp,
            tc.tile_pool(name="smallp", bufs=6) as smallp,
            tc.tile_pool(name="rowp", bufs=2) as rowp,
            tc.tile_pool(name="ps_p", bufs=2, space="PSUM") as ps_p,
            tc.tile_pool(name="ps_o", bufs=2, space="PSUM") as ps_o,
        ):
            identsb = constp.tile([128, 128], F32, name="identsb")
            nc.sync.dma_start(out=identsb[:, :], in_=ident)

            for b in range(B_LOC):
                # ---- natural-layout load: nat[half][p, ii, t] = E[128*(4*half+ii)+p, t]
                nat = []
                enc_b = enc[:, b, :].rearrange("(ii p) t -> p ii t", p=128)
                for half in range(2):
                    natt = natp.tile([128, 4, T], F32, name="natt", tag="nat")
                    nc.sync.dma_start(
                        out=natt[:, :, :], in_=enc_b[:, 4 * half : 4 * half + 4, :]
                    )
                    nat.append(natt)

                def natchunk(i, tsl):
                    return nat[i // 4][:, i % 4, tsl]

                # ---- d_b replicated to all partitions (DMA broadcast from DRAM)
                dbcast = dbp.tile([128, H], F32, name="dbcast", tag="dbcast")
                nc.sync.dma_start(
                    out=dbcast[:, :], in_=dec[b : b + 1, :].to_broadcast([128, H])
                )

                # ---- per t-chunk: transpose, scores, softmax, accumulate output
                o_ps = ps_o.tile([1, H], F32, name="o_ps", tag="ps_o")
                for j in range(NTC):
                    p_ps = ps_p.tile([128, H], F32, name="p_ps", tag="ps_p")
                    for i in range(NHC):
                        # p_ps[t_p, 128*i + h'] = E[128*i + h', 128*j + t_p]
                        nc.tensor.matmul(
                            p_ps[:, 128 * i : 128 * (i + 1)],
                            lhsT=natchunk(i, slice(128 * j, 128 * (j + 1))),
                            rhs=identsb[:, :],
                            is_transpose=True,
                            start=(i % 4 == 0),
                            stop=(i % 4 == 3),
                        )
                    # scores: s_col[t,0] = sum_h Et[t,h]*d[h]   (out write is scratch)
                    junk1 = junkp.tile([128, H], F32, name="junk1", tag="junk")
                    s_col = smallp.tile([128, 1], F32, name="s_col", tag="s_col")
                    nc.vector.scalar_tensor_tensor(
                        out=junk1[:, :],
                        in0=p_ps[:, :],
                        scalar=1.0,
                        in1=dbcast[:, :],
                        op0=OP.mult,
                        op1=OP.mult,
                        accum_out=s_col[:, :],
                    )
                    # mneg = min_h(-s*Et) = -max_h(s*Et)   (out write is scratch)
                    s_neg = smallp.tile([128, 1], F32, name="s_neg", tag="s_neg")
                    nc.vector.tensor_scalar_mul(s_neg[:, :], s_col[:, :], -1.0)
                    junk2 = junkp.tile([128, H], F32, name="junk2", tag="junk")
                    mneg = smallp.tile([128, 1], F32, name="mneg", tag="mneg")
                    nc.vector.tensor_scalar(
                        junk2[:, :],
                        p_ps[:, :],
                        s_neg[:, :],
                        3.0e38,
                        OP.mult,
                        OP.min,
                        accum_out=mneg[:, :],
                    )
                    # e = exp(s*Et - max), z = sum_h e  (z >= 1)
                    e = ep.tile([128, H], final_dt, name="e", tag="e")
                    z = smallp.tile([128, 1], F32, name="z", tag="z")
                    nc.scalar.activation(
                        e[:, :],
                        p_ps[:, :],
                        AF.Exp,
                        bias=mneg[:, :],
                        scale=s_col[:, :],
                        accum_out=z[:, :],
                    )
                    r = smallp.tile([128, 1], F32, name="r", tag="r")
                    nc.vector.reciprocal(r[:, :], z[:, :])
                    rl = smallp.tile([128, 1], final_dt, name="rl", tag="rl")
                    nc.vector.tensor_copy(rl[:, :], r[:, :])
                    # out[0, h] += sum_t r_t * e[t, h]
                    for half in range(2):
                        nc.tensor.matmul(
                            o_ps[0:1, 512 * half : 512 * half + 512],
                            lhsT=rl[:, :],
                            rhs=e[:, 512 * half : 512 * half + 512],
                            start=(j == 0),
                            stop=(j == NTC - 1),
                        )

                orow = rowp.tile([1, H], F32, name="orow", tag="orow")
                nc.scalar.copy(orow[:, :], o_ps[0:1, :])
                nc.sync.dma_start(out=out[b : b + 1, :], in_=orow[:, :])

    if split_waits:
        _split_waits(nc)
    return nc


def make_in_maps(decoder_hidden, encoder_outputs):
    dec = np.ascontiguousarray(np.asarray(decoder_hidden, dtype=np.float32))
    enc = np.ascontiguousarray(np.asarray(encoder_outputs, dtype=np.float32))
    assert dec.shape == (B, H) and enc.shape == (H, B, T)
    ident = np.eye(128, dtype=np.float32)
    in_maps = []
    for k in range(N_CORES):
        bsl = slice(k * B_LOC, (k + 1) * B_LOC)
        in_maps.append(
            {
                "enc": np.ascontiguousarray(enc[:, bsl, :]),
                "dec": np.ascontiguousarray(dec[bsl, :]),
                "ident": ident,
            }
        )
    return in_maps


_PROGRAM = None


def kernel(**inputs) -> np.ndarray:
    global _PROGRAM
    if _PROGRAM is None:
        _PROGRAM = build_program()
    in_maps = make_in_maps(inputs["decoder_hidden"], inputs["encoder_outputs"])
    res = run_bass_kernel_spmd(_PROGRAM, in_maps, core_ids=list(range(N_CORES)))
    return np.concatenate([r["out"] for r in res.results], axis=0)


# revision 13
# speedup vs baseline: 1.5433x; 1.5433x over previous
"""Trainium2 Bass kernel for nn_Attn: batched column-softmax attention energies.

Math (per batch element b):
    E = encoder_outputs[:, b, :]            # [H, T]
    d = decoder_hidden[b]                   # [H]
    s = E^T d                               # [T]  (scores)
    w[h, t] = E[h, t] * s[t]
    sm = softmax over h of w (per column t)
    out[b, h] = sum_t sm[h, t]

Implementation (per core, data parallel over batch: 8 cores x 8 batch):
    - Load E in natural layout [h_part, t_free] as one 4MB DMA per b
      (DMA-efficient: t contiguous per partition).
    - d replicated across partitions once per b via a broadcast DMA
      (partition-stride-0 DRAM source).
    - Per t-chunk j: 8 PE transpose-mode matmuls -> PSUM tile Et [t_part, 1024h]
      (fp32, exact; grouped start/stop per PSUM bank).
    - DVE pass 1 (scalar_tensor_tensor): out = Et * d_bcast (scratch),
      accum_out (sum) = scores s_col [128t, 1] directly per-partition.
    - DVE pass 2 (tensor_scalar, accum op = op1): out = min(-s*Et, 3e38)
      (scratch), accum_out (min) = -max_h(s*Et) = mneg.
    - One ACT op: e = Exp(s_col * Et + mneg) read straight from PSUM, written
      as bf16, accum_out z = sum_h e  (z >= 1 by construction -> no div by 0).
    - DVE reciprocal r = 1/z (+ bf16 cast); PE matmul lhsT=r_bf16:
      out[1, h] += sum_t r_t e[t, h] accumulated over t-chunks in PSUM
      (softmax normalization folded into the matmul).

Container workaround: this walrus build accepts only ONE sync-wait per
instruction; _split_waits() hoists extra waits onto single-wait Drain
carriers after Tile scheduling (cost-model impact ~0.8us). The custom
bass_isa instructions (tensor_tensor_reduce, partition_broadcast) do not
compile here ("ISA wrong length"), so only core mybir instructions are used.
"""

import numpy as np

import concourse.bass as bass
import concourse.mybir as mybir
from concourse.bass_utils import run_bass_kernel_spmd
from concourse.tile import TileContext

H = 1024
B = 64
T = 1024
N_CORES = 8
B_LOC = B // N_CORES  # 8 batch elements per core
NHC = H // 128        # 8 h-chunks
NTC = T // 128        # 8 t-chunks

F32 = mybir.dt.float32
BF16 = mybir.dt.bfloat16


def _split_waits(nc, max_waits=1):
    """Workaround for this container's walrus: control/compute instructions
    accept only one sync-wait command. Hoist extra waits onto single-wait
    Drain carriers inserted just before the instruction (same engine)."""
    n_new = 0
    for f in nc.m.functions:
        for blk in f.blocks:
            new_insts = []
            for inst in blk.instructions:
                si = inst.sync_info
                if si is not None and si.on_wait is not None and len(si.on_wait) > max_waits:
                    waits = list(si.on_wait)
                    while len(waits) > max_waits:
                        w = waits.pop(0)
                        d = mybir.InstDrain(
                            name=f"I-ws-{nc.next_id()}", ins=[], outs=[]
                        )
                        d.engine = inst.engine
                        d.sync_info = mybir.SyncInfo(on_wait=[w], on_update=[])
                        new_insts.append(d)
                        n_new += 1
                    si.on_wait = waits
                new_insts.append(inst)
            blk.instructions = new_insts
    return n_new


def build_program(final_dt=BF16, split_waits=True, stages=6):
    """Build the single-core Bass/Tile program (same program runs SPMD on 8 cores)."""
    nc = bass.Bass("TRN2", debug=False, num_devices=N_CORES)
    enc_h = nc.dram_tensor("enc", [H, B_LOC, T], F32, kind="ExternalInput")
    dec_h = nc.dram_tensor("dec", [B_LOC, H], F32, kind="ExternalInput")
    ident_h = nc.dram_tensor("ident", [128, 128], F32, kind="ExternalInput")
    out_h = nc.dram_tensor("out", [B_LOC, H], F32, kind="ExternalOutput")

    enc = enc_h.ap()
    dec = dec_h.ap()
    ident = ident_h.ap()
    out = out_h.ap()

    AF = mybir.ActivationFunctionType
    OP = mybir.AluOpType

    with TileContext(nc) as tc:
        with (
            tc.tile_pool(name="const", bufs=1) as constp,
            tc.tile_pool(name="natp", bufs=3) as natp,
            tc.tile_pool(name="junkp", bufs=2) as junkp,
            tc.tile_pool(name="ep", bufs=3) as ep,
            tc.tile_pool(name="dbp", bufs=2) as dbp,
            tc.tile_pool(name="smallp", bufs=6) as smallp,
            tc.tile_pool(name="rowp", bufs=2) as rowp,
            tc.tile_pool(name="ps_p", bufs=2, space="PSUM") as ps_p,
            tc.tile_pool(name="ps_o", bufs=2, space="PSUM") as ps_o,
        ):
            identsb = constp.tile([128, 128], F32, name="identsb")
            nc.sync.dma_start(out=identsb[:, :], in_=ident)

            for b in range(B_LOC):
                # ---- natural-layout load: natt[p, i, t] = E[128*i + p, t]  (one 4MB DMA)
                enc_b = enc[:, b, :].rearrange("(ii p) t -> p ii t", p=128)
                natt = natp.tile([128, NHC, T], F32, name="natt", tag="nat")
                nc.sync.dma_start(out=natt[:, :, :], in_=enc_b[:, :, :])

                def natchunk(i, tsl):
                    return natt[:, i, tsl]

                # ---- d_b replicated to all partitions (DMA broadcast from DRAM)
                if stages >= 3:
                    dbcast = dbp.tile([128, H], F32, name="dbcast", tag="dbcast")
                    nc.sync.dma_start(
                        out=dbcast[:, :], in_=dec[b : b + 1, :].to_broadcast([128, H])
                    )

                # ---- per t-chunk: transpose, scores, softmax, accumulate output
                o_ps = ps_o.tile([1, H], F32, name="o_ps", tag="ps_o")
                for j in range(NTC if stages >= 2 else 0):
                    p_ps = ps_p.tile([128, H], F32, name="p_ps", tag="ps_p")
                    for i in range(NHC):
                        # p_ps[t_p, 128*i + h'] = E[128*i + h', 128*j + t_p]
                        nc.tensor.matmul(
                            p_ps[:, 128 * i : 128 * (i + 1)],
                            lhsT=natchunk(i, slice(128 * j, 128 * (j + 1))),
                            rhs=identsb[:, :],
                            is_transpose=True,
                            start=(i % 4 == 0),
                            stop=(i % 4 == 3),
                        )
                    if stages < 3:
                        continue
                    # scores: s_col[t,0] = sum_h Et[t,h]*d[h]   (out write is scratch)
                    junk1 = junkp.tile([128, H], F32, name="junk1", tag="junk")
                    s_col = smallp.tile([128, 1], F32, name="s_col", tag="s_col")
                    nc.vector.scalar_tensor_tensor(
                        out=junk1[:, :],
                        in0=p_ps[:, :],
                        scalar=1.0,
                        in1=dbcast[:, :],
                        op0=OP.mult,
                        op1=OP.mult,
                        accum_out=s_col[:, :],
                    )
                    if stages < 4:
                        continue
                    # mneg = min_h(-s*Et) = -max_h(s*Et)   (out write is scratch)
                    s_neg = smallp.tile([128, 1], F32, name="s_neg", tag="s_neg")
                    nc.vector.tensor_scalar_mul(s_neg[:, :], s_col[:, :], -1.0)
                    junk2 = junkp.tile([128, H], F32, name="junk2", tag="junk")
                    mneg = smallp.tile([128, 1], F32, name="mneg", tag="mneg")
                    nc.vector.tensor_scalar(
                        junk2[:, :],
                        p_ps[:, :],
                        s_neg[:, :],
                        3.0e38,
                        OP.mult,
                        OP.min,
                        accum_out=mneg[:, :],
                    )
                    if stages < 5:
                        continue
                    # e = exp(s*Et - max), z = sum_h e  (z >= 1)
                    e = ep.tile([128, H], final_dt, name="e", tag="e")
                    z = smallp.tile([128, 1], F32, name="z", tag="z")
                    nc.scalar.activation(
                        e[:, :],
                        p_ps[:, :],
                        AF.Exp,
                        bias=mneg[:, :],
                        scale=s_col[:, :],
                        accum_out=z[:, :],
                    )
                    if stages < 6:
                        continue
                    r = smallp.tile([128, 1], F32, name="r", tag="r")
                    nc.vector.reciprocal(r[:, :], z[:, :])
                    rl = smallp.tile([128, 1], final_dt, name="rl", tag="rl")
                    nc.vector.tensor_copy(rl[:, :], r[:, :])
                    # out[0, h] += sum_t r_t * e[t, h]
                    for half in range(2):
                        nc.tensor.matmul(
                            o_ps[0:1, 512 * half : 512 * half + 512],
                            lhsT=rl[:, :],
                            rhs=e[:, 512 * half : 512 * half + 512],
                            start=(j == 0),
                            stop=(j == NTC - 1),
                        )

                if stages >= 6:
                    orow = rowp.tile([1, H], F32, name="orow", tag="orow")
                    nc.scalar.copy(orow[:, :], o_ps[0:1, :])
                    nc.sync.dma_start(out=out[b : b + 1, :], in_=orow[:, :])

    if split_waits:
        _split_waits(nc)
    return nc


def make_in_maps(decoder_hidden, encoder_outputs):
    dec = np.ascontiguousarray(np.asarray(decoder_hidden, dtype=np.float32))
    enc = np.ascontiguousarray(np.asarray(encoder_outputs, dtype=np.float32))
    assert dec.shape == (B, H) and enc.shape == (H, B, T)
    ident = np.eye(128, dtype=np.float32)
    in_maps = []
    for k in range(N_CORES):
        bsl = slice(k * B_LOC, (k + 1) * B_LOC)
        in_maps.append(
            {
                "enc": np.ascontiguousarray(enc[:, bsl, :]),
                "dec": np.ascontiguousarray(dec[bsl, :]),
                "ident": ident,
            }
        )
    return in_maps


_PROGRAM = None


def kernel(**inputs) -> np.ndarray:
    global _PROGRAM
    if _PROGRAM is None:
        _PROGRAM = build_program()
    in_maps = make_in_maps(inputs["decoder_hidden"], inputs["encoder_outputs"])
    res = run_bass_kernel_spmd(_PROGRAM, in_maps, core_ids=list(range(N_CORES)))
    return np.concatenate([r["out"] for r in res.results], axis=0)
